# revision 1
# baseline (speedup 1.0000x reference)
"""Trainium2 Bass kernel for nn_DecoderRNN (Bahdanau-attention GRU decoder).

v3: pure data parallel over batch (128 -> 16 per core, 8 cores), bf16 matmuls
with f32 PSUM accumulation.

Layout change vs v2: attention tensors live in [h_tile(128p), n(196), b(16)]
order so the hq broadcast-add runs in DVE 2x mode as ONE instruction per
h-tile, and scores become M=1 matmuls over (n,b) columns. whhT/whT/featp are
SBUF-resident (wxT streams in quarter tiles; proj streams in 4-way-split
chunk DMAs under the attention window). h history stays in SBUF for the
end-of-sequence classifier.
"""
import os
import sys

sys.path.insert(0, "/opt/trn_rl_repo")

import numpy as np
import ml_dtypes

import concourse.bass as bass
import concourse.tile as tile
from concourse import mybir
from concourse.bass_utils import run_bass_kernel_spmd
from concourse.masks import make_identity

F32 = mybir.dt.float32
BF16 = mybir.dt.bfloat16
FP8 = mybir.dt.float8e4
bf = ml_dtypes.bfloat16
AL = mybir.AluOpType
AF = mybir.ActivationFunctionType

NCORES = 8
B = 16            # local batch per core
N = 196           # attention positions
H = 1024          # hidden
E = 512           # embed dim
G = 3 * H         # gate width
T = int(os.environ.get("DECODER_STEPS", "17"))
C = 1000          # classes
NB = N * B        # 3136 ((n,b) cols)
KH = 8            # h k-tiles (1024/128)
KB = 32           # padded (b,n) k-tiles for ctx (16*256/128)
SCW = 448         # scores chunk width ((n,b) cols; 28 n x 16 b)
NSC = 7           # scores chunks (7*448 = 3136)
CT = 8            # classifier m-tiles (1024 padded)
TB = T * B


_CACHE = {}


def _split_waits(nc, keep=1):
    """This container's walrus build rejects >1 sem-wait per instruction
    (setupSyncWait: 'Too many sync wait commands'). Hoist all but one wait
    of every instruction onto single-wait NoOps on the same engine, placed
    immediately before it in program order."""
    nfix = 0
    for bb in nc.main_func.blocks:
        il = bb.instructions
        i = 0
        while i < len(il):
            ins = il[i]
            si = getattr(ins, 'sync_info', None)
            if si is not None and len(si.on_wait) > keep:
                waits = list(si.on_wait)
                for w_i, w in enumerate(waits[:-keep]):
                    nop = mybir.InstNoOp(name=f"{ins.name}-ws{w_i}", ins=[],
                                         outs=[])
                    nop.engine = ins.engine
                    nop.sync_info = mybir.SyncInfo(on_wait=[w], on_update=[])
                    il.insert(i, nop)
                    i += 1
                ins.sync_info = mybir.SyncInfo(on_wait=waits[-keep:],
                                               on_update=list(si.on_update))
                nfix += 1
            i += 1
    return nfix


def _build_program():
    nc = bass.Bass()

    featp_d = nc.declare_dram_parameter("featp", [KB, 128, H], BF16, isOutput=False)
    featT_d = nc.declare_dram_parameter("featT", [KH, 128, NB], BF16, isOutput=False)
    wcT_d = nc.declare_dram_parameter("wcT", [KH, 128, H], BF16, isOutput=False)
    wxT_d = nc.declare_dram_parameter("wxT", [KH, 128, G], BF16, isOutput=False)
    whhT_d = nc.declare_dram_parameter("whhT", [KH, 128, G], BF16, isOutput=False)
    whT_d = nc.declare_dram_parameter("whT", [KH, 128, H], BF16, isOutput=False)
    wclsT_d = nc.declare_dram_parameter("wclsT", [KH, 128, CT * 128], BF16, isOutput=False)
    vT_d = nc.declare_dram_parameter("vT", [KH, 128, 1], BF16, isOutput=False)
    ge_d = nc.declare_dram_parameter("ge", [T, B, G], BF16, isOutput=False)
    h0b_d = nc.declare_dram_parameter("h0b", [B, H], F32, isOutput=False)
    hpk0_d = nc.declare_dram_parameter("hpk0", [128, 128], BF16, isOutput=False)
    bhpk_d = nc.declare_dram_parameter("bhpk", [128, 128], BF16, isOutput=False)
    bc_d = nc.declare_dram_parameter("bc", [1, H], BF16, isOutput=False)
    out_d = nc.declare_dram_parameter("out", [CT, 128, TB], F32, isOutput=True)

    # per-step attention input, [h_out_tile, 128, (n,b)] in (n,b) order
    projT_d = nc.dram_tensor("projT", [KH, 128, NB], BF16)

    with tile.TileContext(nc) as tc:
        with tc.tile_pool(name="persist", bufs=1) as P1, \
             tc.tile_pool(name="state", bufs=2) as P2:

            # ---- small persistent tensors
            whT_s = P1.tile([128, KH, H], BF16)
            vT_s = P1.tile([128, KH], BF16)
            for k in range(KH):
                nc.sync.dma_start(whT_s[:, k, :], whT_d[k])
                nc.sync.dma_start(vT_s[:, k:k + 1], vT_d[k])
            ident16 = P1.tile([B, B], BF16)
            make_identity(nc, ident16)
            bhpk_s = P1.tile([128, 128], BF16)
            nc.sync.dma_start(bhpk_s, bhpk_d[:])
            wblk = P1.tile([128, 33 * B], BF16)
            nc.vector.memset(wblk, 0.0)
            hs_sb = P1.tile([128, KH, T, B], BF16)   # h history for classifier

            h32 = P2.tile([B, H], F32, tag="h32")
            nc.sync.dma_start(h32, h0b_d[:])
            hpk = P2.tile([128, 128], BF16, tag="hpk")
            nc.sync.dma_start(hpk, hpk0_d[:])

            # ---- startup: cnn_proj = feat @ Wc^T + bc -> projT_d (HBM)
            # out layout [h_out tile m, 128, (n,b)]; featT loaded once and
            # kept resident; m-outer so proj[0] completes early for step 0
            with tc.tile_pool(name="wcpool", bufs=1) as Pwc, \
                 tc.tile_pool(name="stage", bufs=3) as Pstage, \
                 tc.tile_pool(name="ps_start", bufs=3, space="PSUM") as PSs:
                wcT_s = Pwc.tile([128, KH, H], BF16)
                ones448 = Pwc.tile([1, SCW], BF16)
                nc.vector.memset(ones448, 1.0)
                bc_s = Pwc.tile([1, H], BF16)
                nc.sync.dma_start(bc_s, bc_d[:])
                for k in range(KH):
                    nc.sync.dma_start(wcT_s[:, k, :], wcT_d[k])
                ft_s = Pwc.tile([128, KH, NB], BF16)
                for k in range(KH):
                    for q in range(4):
                        ql = slice(q * (NB // 4), (q + 1) * (NB // 4))
                        nc.sync.dma_start(ft_s[:, k, ql], featT_d[k][:, ql])
                for m in range(KH):
                    for cch in range(NSC):
                        sl = slice(cch * SCW, (cch + 1) * SCW)
                        ps = PSs.tile([128, SCW], F32, tag="ps",
                                      name=f"ps{m}_{cch}")
                        nc.tensor.matmul(
                            ps, bc_s[0:1, m * 128:(m + 1) * 128], ones448,
                            start=True, stop=False)
                        for k in range(KH):
                            nc.tensor.matmul(
                                ps, wcT_s[:, k, m * 128:(m + 1) * 128],
                                ft_s[:, k, sl],
                                start=False, stop=(k == KH - 1))
                        st = Pstage.tile([128, SCW], BF16, tag="st",
                                         name=f"st{m}_{cch}")
                        if cch % 2 == 0:
                            nc.vector.tensor_copy(st, ps)
                        else:
                            nc.scalar.activation(st, ps, AF.Copy)
                        nc.sync.dma_start(projT_d[m][:, sl], st)

            # ---- big resident weights (space freed by startup pool)
            with tc.tile_pool(name="wres", bufs=1) as Pw, \
                 tc.tile_pool(name="projring", bufs=2) as Pstr, \
                 tc.tile_pool(name="wxring", bufs=3) as Pwx, \
                 tc.tile_pool(name="gering", bufs=1) as Pge, \
                 tc.tile_pool(name="small", bufs=1) as Psm, \
                 tc.tile_pool(name="gt", bufs=2) as Pgt, \
                 tc.tile_pool(name="gf", bufs=1) as Pgf:
                whhT_s = Pw.tile([128, KH, G], BF16)
                for k in range(KH):
                    for q in range(4):
                        ql = slice(q * (G // 4), (q + 1) * (G // 4))
                        nc.sync.dma_start(whhT_s[:, k, ql], whhT_d[k][:, ql])
                feat_s = Pw.tile([128, KB, H], BF16)
                for kb in range(KB):
                    nc.sync.dma_start(feat_s[:, kb, :], featp_d[kb])

                for t in range(T):
                    ge_t = Pge.tile([B, G], BF16, tag="ge", name=f"ge{t}")
                    nc.sync.dma_start(ge_t, ge_d[t])
                    # wx stream ring: full k-tiles [128, 3072], ring 3
                    wx_tiles = []
                    for k in range(KH):
                        wx_tiles.append(Pwx.tile([128, G], BF16, tag="wx",
                                                 name=f"wx{t}_{k}"))
                    for k in range(3):
                        nc.gpsimd.dma_start(wx_tiles[k], wxT_d[k])

                    # ---- hq (packed via transposes, bias folded) -> hq_sb
                    hqf = Psm.tile([B, H], BF16, tag="hqf", name=f"hqf{t}")
                    hq_sb = Psm.tile([128, 128], BF16, tag="hqsb",
                                     name=f"hqsb{t}", bufs=2)
                    with tc.tile_pool(name="psA", bufs=1, space="PSUM") as PA, \
                         tc.tile_pool(name="psT", bufs=2, space="PSUM") as PT:
                        pqs = [PA.tile([B, 512], F32, tag=f"hqp{c}",
                                       name=f"hqp{t}_{c}") for c in range(2)]
                        for k in range(KH):
                            for c in range(2):
                                nc.tensor.matmul(
                                    pqs[c], hpk[:, k * B:(k + 1) * B],
                                    whT_s[:, k, c * 512:(c + 1) * 512],
                                    start=(k == 0), stop=(k == KH - 1))
                        for c in range(2):
                            nc.vector.tensor_copy(
                                hqf[:, c * 512:(c + 1) * 512], pqs[c])
                        for m in range(KH):
                            tp = PT.tile([128, B], BF16, tag="tphq",
                                         name=f"tphq{t}_{m}")
                            nc.tensor.transpose(
                                tp, hqf[:, m * 128:(m + 1) * 128], ident16)
                            nc.vector.scalar_tensor_tensor(
                                out=hq_sb[:, m * B:(m + 1) * B], in0=tp,
                                scalar=1.0, in1=bhpk_s[:, m * B:(m + 1) * B],
                                op0=AL.mult, op1=AL.add)

                    # ---- attention window: per h-tile add+tanh+scores,
                    # gh interleaved on PE between scores bursts
                    ghge = Psm.tile([B, 2 * H], BF16, tag="ghge",
                                    name=f"ghge{t}")
                    hn_sb = Psm.tile([B, H], BF16, tag="hn", name=f"hn{t}")
                    scflat = Pstr.tile([1, NB], BF16, tag="x",
                                        name=f"scflat{t}")
                    scores_sb = Psm.tile([B, N], BF16, tag="scores",
                                         name=f"scores{t}")
                    with tc.tile_pool(name="psB", bufs=1, space="PSUM") as PB, \
                         tc.tile_pool(name="psG", bufs=1, space="PSUM") as PG:
                        sc_ps = [PB.tile([1, SCW], F32, tag=f"sc{c}",
                                         name=f"sc{t}_{c}")
                                 for c in range(NSC)]

                        def gh_chunk(c):
                            ps = PG.tile([B, 512], F32, tag="ghp",
                                         name=f"ghp{t}_{c}")
                            for k in range(KH):
                                nc.tensor.matmul(
                                    ps, hpk[:, k * B:(k + 1) * B],
                                    whhT_s[:, k, c * 512:(c + 1) * 512],
                                    start=(k == 0), stop=(k == KH - 1))
                            if c < 4:
                                nc.vector.scalar_tensor_tensor(
                                    out=ghge[:, c * 512:(c + 1) * 512],
                                    in0=ps, scalar=0.5,
                                    in1=ge_t[:, c * 512:(c + 1) * 512],
                                    op0=AL.mult, op1=AL.add)
                            else:
                                nc.scalar.activation(
                                    hn_sb[:, (c - 4) * 512:(c - 3) * 512],
                                    ps, AF.Copy)

                        ghq = list(range(6))
                        for hi in range(KH):
                            xc = Pstr.tile([128, NB], BF16, tag="x",
                                           name=f"x{t}_{hi}")
                            nc.gpsimd.dma_start(xc, projT_d[hi])
                            x3 = xc.rearrange("p (n b) -> p n b", b=B)
                            hqb = hq_sb[:, hi * B:(hi + 1) * B] \
                                .unsqueeze(1).broadcast_to([128, N, B])
                            nc.vector.tensor_tensor(out=x3, in0=x3, in1=hqb,
                                                    op=AL.add)
                            nc.scalar.activation(xc, xc, AF.Tanh)
                            for c in range(NSC):
                                nc.tensor.matmul(
                                    sc_ps[c], vT_s[:, hi:hi + 1],
                                    xc[:, c * SCW:(c + 1) * SCW],
                                    start=(hi == 0), stop=(hi == KH - 1))
                            if 1 <= hi <= 6:
                                gh_chunk(ghq.pop(0))

                        # evacuate score chunks (PSUM -> 1-partition SBUF)
                        # with an (n,b)->(b,n) permute inside the copy, then
                        # one balanced reshape DMA to [b, n]
                        sc_bn = scflat.rearrange("o (b n) -> o b n", n=N)
                        for c in range(NSC):
                            seg = sc_bn[:, :, c * (SCW // B):
                                        (c + 1) * (SCW // B)]
                            src = sc_ps[c].rearrange("o (n b) -> o b n", b=B)
                            if c % 2 == 0:
                                nc.vector.tensor_copy(seg, src)
                            else:
                                nc.scalar.activation(seg, src, AF.Copy)
                    nc.sync.dma_start(
                        out=scores_sb,
                        in_=scflat.rearrange("o (b n) -> o b n", n=N))

                    # ---- softmax (late-normalized: exps stay unnormalized)
                    exps = Psm.tile([B, N], BF16, tag="exps", name=f"exps{t}")
                    sumexp = Psm.tile([B, 1], F32, tag="sumexp",
                                      name=f"sumexp{t}")
                    nc.scalar.activation(exps, scores_sb, AF.Exp,
                                         accum_out=sumexp)
                    rec = Psm.tile([B, 1], F32, tag="rec", name=f"rec{t}")
                    nc.vector.reciprocal(rec, sumexp)
                    wT_sb = Psm.tile([128, 2 * B], BF16, tag="wT",
                                     name=f"wT{t}", bufs=2)
                    with tc.tile_pool(name="psW", bufs=2, space="PSUM") as PW:
                        wt0 = PW.tile([128, B], BF16, tag="wt0",
                                      name=f"wt0{t}")
                        nc.tensor.transpose(wt0, exps[:, 0:128], ident16)
                        nc.vector.tensor_copy(wT_sb[:, 0:B], wt0)
                        wt1 = PW.tile([68, B], BF16, tag="wt1",
                                      name=f"wt1{t}")
                        nc.tensor.transpose(wt1, exps[:, 128:196], ident16)
                        nc.vector.tensor_copy(wT_sb[0:68, B:2 * B], wt1)
                    wv = wblk.rearrange("p (b r) -> p b r", r=33)
                    nc.sync.dma_start(out=wv[:, :, 0:1],
                                      in_=wT_sb[:, 0:B].unsqueeze(2))
                    nc.sync.dma_start(out=wv[0:68, :, 16:17],
                                      in_=wT_sb[0:68, B:2 * B].unsqueeze(2))

                    # ---- ctx (unnormalized accumulate, scale by rec on evac)
                    ctxs = Psm.tile([B, H], BF16, tag="hqf", name=f"ctxs{t}")
                    ctxT = Psm.tile([128, 128], BF16, tag="ctxT",
                                    name=f"ctxT{t}")
                    with tc.tile_pool(name="psC", bufs=1, space="PSUM") as PC:
                        ctxL = PC.tile([B, 512], F32, tag="ctxL",
                                       name=f"ctxL{t}")
                        ctxR = PC.tile([B, 512], F32, tag="ctxR",
                                       name=f"ctxR{t}")
                        for kb in range(KB):
                            lhs = wblk[:, kb * B:(kb + 1) * B]
                            nc.tensor.matmul(ctxL, lhs, feat_s[:, kb, 0:512],
                                             start=(kb == 0),
                                             stop=(kb == KB - 1))
                            nc.tensor.matmul(ctxR, lhs,
                                             feat_s[:, kb, 512:1024],
                                             start=(kb == 0),
                                             stop=(kb == KB - 1))
                        nc.vector.tensor_scalar(
                            out=ctxs[:, 0:512], in0=ctxL, scalar1=rec,
                            scalar2=None, op0=AL.mult)
                        nc.vector.tensor_scalar(
                            out=ctxs[:, 512:1024], in0=ctxR, scalar1=rec,
                            scalar2=None, op0=AL.mult)
                    with tc.tile_pool(name="psT2", bufs=2,
                                      space="PSUM") as PT2:
                        for m in range(KH):
                            tp2 = PT2.tile([128, B], BF16, tag="tpc",
                                           name=f"tpc{t}_{m}")
                            nc.tensor.transpose(
                                tp2, ctxs[:, m * 128:(m + 1) * 128], ident16)
                            nc.vector.tensor_copy(
                                ctxT[:, m * B:(m + 1) * B], tp2)

                    # ---- gi (wxT streamed ring) + gate evac
                    for k in range(3, KH):
                        nc.gpsimd.dma_start(wx_tiles[k], wxT_d[k])
                    srz = Psm.tile([B, 2 * H], BF16, tag="srz",
                                   name=f"srz{t}")
                    nin = Psm.tile([B, H], BF16, tag="nin", name=f"nin{t}")
                    with tc.tile_pool(name="psGI", bufs=1, space="PSUM") as PGi:
                        gps = [PGi.tile([B, 512], F32, tag=f"gi{c}",
                                        name=f"gi{t}_{c}") for c in range(6)]
                        for k in range(KH):
                            for c in range(6):
                                nc.tensor.matmul(
                                    gps[c], ctxT[:, k * B:(k + 1) * B],
                                    wx_tiles[k][:, c * 512:(c + 1) * 512],
                                    start=(k == 0), stop=(k == KH - 1))
                        for c in range(4):
                            nc.vector.scalar_tensor_tensor(
                                out=srz[:, c * 512:(c + 1) * 512], in0=gps[c],
                                scalar=0.5,
                                in1=ghge[:, c * 512:(c + 1) * 512],
                                op0=AL.mult, op1=AL.add)
                        for c in range(2):
                            nc.vector.scalar_tensor_tensor(
                                out=nin[:, c * 512:(c + 1) * 512],
                                in0=gps[4 + c], scalar=1.0,
                                in1=ge_t[:, 2 * H + c * 512:
                                         2 * H + (c + 1) * 512],
                                op0=AL.mult, op1=AL.add)

                    # ---- GRU elementwise ([16, *] layout)
                    nc.scalar.activation(srz, srz, AF.Tanh)
                    r_ = Pgt.tile([B, H], BF16, tag="gt", name=f"r{t}")
                    nc.vector.tensor_scalar(out=r_, in0=srz[:, 0:H],
                                            scalar1=0.5, scalar2=0.5,
                                            op0=AL.mult, op1=AL.add)
                    rhn = Pgt.tile([B, H], BF16, tag="gt", name=f"rhn{t}")
                    nc.gpsimd.tensor_tensor(out=rhn, in0=r_, in1=hn_sb,
                                            op=AL.mult)
                    narg = Pgt.tile([B, H], BF16, tag="gt", name=f"narg{t}")
                    nc.vector.tensor_tensor(out=narg, in0=rhn, in1=nin,
                                            op=AL.add)
                    n_ = Pgf.tile([B, H], F32, tag="gf", name=f"n{t}")
                    nc.scalar.activation(n_, narg, AF.Tanh)
                    z_ = Pgt.tile([B, H], BF16, tag="gt", name=f"z{t}")
                    nc.gpsimd.tensor_scalar(out=z_, in0=srz[:, H:2 * H],
                                            scalar1=0.5, scalar2=0.5,
                                            op0=AL.mult, op1=AL.add)
                    d_ = Pgf.tile([B, H], F32, tag="gfd", name=f"d{t}", bufs=1)
                    nc.vector.tensor_tensor(out=d_, in0=h32, in1=n_,
                                            op=AL.subtract)
                    zd = Pgt.tile([B, H], BF16, tag="gt", name=f"zd{t}")
                    nc.gpsimd.tensor_tensor(out=zd, in0=z_, in1=d_,
                                            op=AL.mult)
                    h32n = P2.tile([B, H], F32, tag="h32", name=f"h32_{t}")
                    nc.vector.tensor_tensor(out=h32n, in0=n_, in1=zd,
                                            op=AL.add)
                    h16f = Pgt.tile([B, H], BF16, tag="gt", name=f"h16f{t}")
                    nc.vector.tensor_copy(h16f, h32n)
                    hpk_n = P2.tile([128, 128], BF16, tag="hpk",
                                    name=f"hpk{t}")
                    with tc.tile_pool(name="psT3", bufs=2,
                                      space="PSUM") as PT3:
                        for m in range(KH):
                            tp3 = PT3.tile([128, B], BF16, tag="tph",
                                           name=f"tph{t}_{m}")
                            nc.tensor.transpose(
                                tp3, h16f[:, m * 128:(m + 1) * 128], ident16)
                            if m % 2 == 0:
                                nc.vector.tensor_copy(
                                    hpk_n[:, m * B:(m + 1) * B], tp3)
                                nc.scalar.activation(
                                    hs_sb[:, m, t, :], tp3, AF.Copy)
                            else:
                                nc.scalar.activation(
                                    hpk_n[:, m * B:(m + 1) * B], tp3, AF.Copy)
                                nc.vector.tensor_copy(
                                    hs_sb[:, m, t, :], tp3)
                    h32, hpk = h32n, hpk_n

            # ---- classifier from SBUF h history
            with tc.tile_pool(name="clsw", bufs=1) as Pc, \
                 tc.tile_pool(name="outst", bufs=2) as Po, \
                 tc.tile_pool(name="psE", bufs=2, space="PSUM") as PEp:
                wcls_s = Pc.tile([128, KH, CT * 128], BF16)
                for k in range(KH):
                    for q in range(4):
                        ql = slice(q * CT * 32, (q + 1) * CT * 32)
                        nc.sync.dma_start(wcls_s[:, k, ql],
                                          wclsT_d[k][:, ql])
                for mc in range(CT):
                    ps = PEp.tile([128, TB], F32, tag="cls", name=f"cls{mc}")
                    for k in range(KH):
                        nc.tensor.matmul(
                            ps,
                            wcls_s[:, k, mc * 128:(mc + 1) * 128],
                            hs_sb[:, k, :, :],
                            start=(k == 0), stop=(k == KH - 1))
                    ot = Po.tile([128, TB], F32, tag="ot", name=f"ot{mc}")
                    if mc % 2 == 0:
                        nc.vector.tensor_copy(ot, ps)
                    else:
                        nc.scalar.activation(ot, ps, AF.Copy)
                    nc.sync.dma_start(out_d[mc], ot)

    _split_waits(nc)
    return nc


def _get_program():
    if "nc" not in _CACHE:
        _CACHE["nc"] = _build_program()
    return _CACHE["nc"]


def _pack_inputs(cnn_feat, labels, sos, h0, embed_table, W_ih, b_ih, W_hh,
                 b_hh, Wh, bh, Wc, bc, v_w, Wcls):
    """Host-side layout prep. Returns list of per-core input dicts."""
    f32 = np.float32
    cnn_feat = np.asarray(cnn_feat, f32)
    labels = np.asarray(labels)
    W_ih = np.asarray(W_ih, f32)
    We = W_ih[:, :E]                     # [G, E]
    Wx = W_ih[:, E:]                     # [G, H]

    Ball = cnn_feat.shape[0]
    emb = np.asarray(embed_table, f32)[labels]               # [128, 17, E]
    emb_in = np.concatenate(
        [np.broadcast_to(np.asarray(sos, f32), (Ball, 1, E)), emb],
        axis=1)[:, :T]
    geh = emb_in @ We.T + np.asarray(b_ih, f32) + np.asarray(b_hh, f32)
    geh[..., :2 * H] *= 0.5              # pre-halve r,z parts  [128, T, G]

    wcT = np.ascontiguousarray(np.asarray(Wc, f32).T).reshape(KH, 128, H).astype(bf)
    wxT = np.ascontiguousarray(Wx.T).reshape(KH, 128, G).astype(bf)
    whhT = np.ascontiguousarray(np.asarray(W_hh, f32).T).reshape(KH, 128, G).astype(bf)
    whT = np.ascontiguousarray(np.asarray(Wh, f32).T).reshape(KH, 128, H).astype(bf)
    wcls_pad = np.zeros((CT * 128, H), f32)
    wcls_pad[:C] = np.asarray(Wcls, f32)
    wclsT = np.ascontiguousarray(wcls_pad.T).reshape(KH, 128, CT * 128).astype(bf)
    vT = np.ascontiguousarray(
        np.asarray(v_w, f32).reshape(KH, 128, 1)).astype(bf)
    h0 = np.asarray(h0, f32)
    h0b = np.ascontiguousarray(np.broadcast_to(h0, (B, H)), f32)
    hpk0 = np.ascontiguousarray(np.broadcast_to(
        h0.reshape(KH, 128, 1), (KH, 128, B)).transpose(1, 0, 2).reshape(128, 128)).astype(bf)
    bh_a = np.asarray(bh, f32)
    bhpk = np.ascontiguousarray(np.broadcast_to(
        bh_a.reshape(KH, 128, 1), (KH, 128, B)).transpose(1, 0, 2).reshape(128, 128)).astype(bf)
    bc_a = np.asarray(bc, f32).reshape(1, H).astype(bf)

    in_maps = []
    for core in range(NCORES):
        b0 = core * B
        fc = cnn_feat[b0:b0 + B]                     # [16, 196, 1024]
        featp = np.zeros((B, 256, H), f32)
        featp[:, :N, :] = fc
        featp = featp.reshape(KB, 128, H).astype(bf)
        # featT in (h, n, b) order for the (n,b)-column attention layout
        featT = np.ascontiguousarray(
            fc.transpose(2, 1, 0).reshape(H, NB)).reshape(KH, 128, NB).astype(bf)
        gepack = np.ascontiguousarray(
            geh[b0:b0 + B].transpose(1, 0, 2)).astype(bf)    # [T, B, G]
        in_maps.append({
            "featp": featp,
            "featT": featT,
            "wcT": wcT,
            "wxT": wxT,
            "whhT": whhT,
            "whT": whT,
            "wclsT": wclsT,
            "vT": vT,
            "ge": gepack,
            "h0b": h0b,
            "hpk0": hpk0,
            "bhpk": bhpk,
            "bc": bc_a,
        })
    return in_maps


def kernel(cnn_feat, labels, lens, sos, h0, embed_table, W_ih, b_ih, W_hh,
           b_hh, Wh, bh, Wc, bc, v_w, v_b, Wcls, bcls):
    # v_b shifts all scores uniformly -> softmax-invariant -> dropped.
    nc = _get_program()
    in_maps = _pack_inputs(cnn_feat, labels, sos, h0, embed_table, W_ih, b_ih,
                           W_hh, b_hh, Wh, bh, Wc, bc, v_w, Wcls)
    res = run_bass_kernel_spmd(nc, in_maps, list(range(NCORES)))
    outs = []
    bcls = np.asarray(bcls, np.float32)
    for core in range(NCORES):
        o = np.asarray(res.results[core]["out"], np.float32)  # [CT,128,TB]
        o = o.reshape(CT * 128, T, B)                         # [1024, T, B]
        o = o[:C].transpose(2, 1, 0)                          # [B, T, C]
        outs.append(o)
    full = np.concatenate(outs, axis=0) + bcls                # [128, T, C]
    return np.ascontiguousarray(full, np.float32)


if __name__ == "__main__":
    rng = np.random.default_rng(0)
    s = 0.02
    inputs = dict(
        cnn_feat=rng.standard_normal((128, N, H), dtype=np.float32),
        labels=rng.integers(0, C, (128, 17)).astype(np.int32),
        lens=rng.integers(1, 17, (128,)).astype(np.int32),
        sos=(rng.standard_normal(E) * s).astype(np.float32),
        h0=(rng.standard_normal(H) * s).astype(np.float32),
        embed_table=(rng.standard_normal((C, E)) * s).astype(np.float32),
        W_ih=(rng.standard_normal((G, E + H)) * s).astype(np.float32),
        b_ih=np.zeros(G, np.float32),
        W_hh=(rng.standard_normal((G, H)) * s).astype(np.float32),
        b_hh=np.zeros(G, np.float32),
        Wh=(rng.standard_normal((H, H)) * s).astype(np.float32),
        bh=np.zeros(H, np.float32),
        Wc=(rng.standard_normal((H, H)) * s).astype(np.float32),
        bc=np.zeros(H, np.float32),
        v_w=(rng.standard_normal(H) * s).astype(np.float32),
        v_b=np.zeros((), np.float32),
        Wcls=(rng.standard_normal((C, H)) * s).astype(np.float32),
        bcls=np.zeros(C, np.float32),
    )
    out = kernel(**inputs)
    print("out", out.shape, out.dtype, float(np.abs(out).max()))

